# revision 27
# baseline (speedup 1.0000x reference)
"""Trainium2 Bass kernel for nn_AttentionNetwork (ragged path attention).

Data-parallel over 8 NeuronCores: 512 paths per core. Paths are sorted by
length (host-side) and packed into variable-width blocks (bp paths x cap
node-slots, bp*cap <= 1024, cap = max length in the block; capacities are
taken as the element-wise max over cores so one SPMD program serves all
8). This skips the ~45% of node slots beyond each path's length that a
fixed 64-slot layout would waste. Per block: node-MLP in float32r
(TF32-like PE mode: full bf16 throughput, ~1.5e-4 matmul error),
length-masked softmax over nodes (additive mask folded into the
score-matmul PSUM group as a K=1 accumulate; exp row broadcast across
partitions on GpSimd), then the softmax-weighted node sum on the vector
engine from a bf16 copy of X. Stage 2 (f32r) computes path-attention
scores and returns exp-weighted partial sums + (max, sumexp) stats; the
host combines the 8 partials (softmax over paths is permutation-
invariant, so the sorted order needs no undoing).
"""

import sys

if "/opt/trn_rl_repo" not in sys.path:
    sys.path.insert(0, "/opt/trn_rl_repo")

from contextlib import ExitStack

import ml_dtypes
import numpy as np

import concourse.bass as bass  # noqa: F401
import concourse.mybir as mybir
import concourse.tile as tile
from concourse import bacc, bass_utils

P, LMAX, D, H = 4096, 64, 512, 512
NCORES = 8
PS = P // NCORES          # paths per core
KC = D // 128             # contraction chunks
HC = H // 128             # hidden tiles
MASK_NEG = -30000.0
ROWS_TARGET = 1024        # max rows (bp*cap) per block

f32 = mybir.dt.float32
f32r = mybir.dt.float32r
bf16 = mybir.dt.bfloat16
AF = mybir.ActivationFunctionType
ALU = mybir.AluOpType
AX = mybir.AxisListType

LAST_RESULT = None
_PROG_CACHE = {}
_TRACE_KW = {}


def _make_blocks(len_max):
    """Greedy pack sorted-desc lengths into (bp, cap) blocks, bp*cap<=1024."""
    blocks = []
    i = 0
    while i < PS:
        cap = int(len_max[i])
        bp = min(ROWS_TARGET // cap, PS - i)
        if (bp * cap) % 2:
            cap += 1          # keep matmul free dims even (fp32r ISA rule)
        blocks.append((bp, cap))
        i += bp
    return tuple(blocks)


def _build_program(blocks, stage="full"):
    """blocks: tuple of (bp, cap); one block = bp paths x cap node slots."""
    nb = len(blocks)
    rows_list = [bp * cap for bp, cap in blocks]
    tot_rows = sum(rows_list)

    nc = bacc.Bacc("TRN2", target_bir_lowering=False, debug=False, num_devices=NCORES)

    xb = nc.dram_tensor("xb", [KC * 128 * tot_rows], bf16, kind="ExternalInput")
    msk = nc.dram_tensor("msk", [tot_rows], bf16, kind="ExternalInput")
    w1 = nc.dram_tensor("w1", [KC, 128, H], f32r, kind="ExternalInput")
    w2 = nc.dram_tensor("w2", [128, HC], f32r, kind="ExternalInput")
    b1 = nc.dram_tensor("b1", [128, HC], f32, kind="ExternalInput")
    aw1 = nc.dram_tensor("aw1", [KC, 128, H], f32r, kind="ExternalInput")
    ab1 = nc.dram_tensor("ab1", [128, HC], f32, kind="ExternalInput")
    aw2 = nc.dram_tensor("aw2", [128, HC], f32r, kind="ExternalInput")
    one1_bf = nc.dram_tensor("one1_bf", [1, 1], bf16, kind="ExternalInput")
    out_part = nc.dram_tensor("out_part", [128, KC], f32, kind="ExternalOutput")
    out_stats = nc.dram_tensor("out_stats", [1, 2], f32, kind="ExternalOutput")
    dbg = None
    if stage != "full":
        dbg = nc.dram_tensor("dbg", [128, KC, PS], f32, kind="ExternalOutput")

    with ExitStack() as ctx:
        tc = ctx.enter_context(tile.TileContext(nc))
        const = ctx.enter_context(tc.tile_pool(name="const", bufs=1))
        xpool = ctx.enter_context(tc.tile_pool(name="x", bufs=3))
        xwpool = ctx.enter_context(tc.tile_pool(name="xw", bufs=2))
        hpool = ctx.enter_context(tc.tile_pool(name="h", bufs=2))
        wpool = ctx.enter_context(tc.tile_pool(name="w", bufs=2))
        spool = ctx.enter_context(tc.tile_pool(name="s", bufs=3))
        ph_pool = ctx.enter_context(tc.tile_pool(name="ph", bufs=6, space="PSUM"))
        ps_pool = ctx.enter_context(tc.tile_pool(name="ps", bufs=2, space="PSUM"))

        t_w1 = const.tile([128, KC, H], f32r)
        nc.sync.dma_start(t_w1[:], w1.ap().rearrange("k d h -> d k h"))
        t_w2 = const.tile([128, HC], f32r)
        nc.sync.dma_start(t_w2[:], w2.ap())
        t_b1 = const.tile([128, HC], f32)
        nc.sync.dma_start(t_b1[:], b1.ap())
        t_one1 = const.tile([1, 1], bf16)
        nc.sync.dma_start(t_one1[:], one1_bf.ap())
        # ACT table prefetch: force the exp_and_others load before data arrives
        t_warm = const.tile([1, 1], f32)
        nc.scalar.activation(t_warm[:], t_one1[:], AF.Exp)
        t_aw1 = const.tile([128, KC, H], f32r)
        t_ab1 = const.tile([128, HC], f32)
        t_aw2 = const.tile([128, HC], f32r)

        pfT = const.tile([128, KC, PS], f32r)  # normalized path features

        x_offs = [0] * nb
        m_offs = [0] * nb
        p_offs = [0] * nb
        acc_x = acc_m = acc_p = 0
        for i in range(nb):
            x_offs[i], m_offs[i], p_offs[i] = acc_x, acc_m, acc_p
            acc_x += KC * 128 * rows_list[i]
            acc_m += rows_list[i]
            acc_p += blocks[i][0]
        assert acc_p == PS

        emit_order = [nb - 1] + list(range(nb - 1))
        for ei, b in enumerate(emit_order):
            bp, cap = blocks[b]
            rows = rows_list[b]
            x_off, m_off, p_off = x_offs[b], m_offs[b], p_offs[b]

            x_b = xpool.tile([128, KC, rows], bf16, tag="xb", name=f"xb_{b}")
            nc.sync.dma_start(
                x_b[:],
                xb.ap()[x_off : x_off + KC * 128 * rows].rearrange(
                    "(k d r) -> d k r", k=KC, d=128
                ),
            )
            x_r = xpool.tile([128, KC, rows], f32r, tag="xr", name=f"xr_{b}")
            nc.scalar.copy(x_r[:, 0:2, :], x_b[:, 0:2, :])
            nc.vector.tensor_copy(x_r[:, 2:4, :], x_b[:, 2:4, :])
            mrow = spool.tile([1, rows], bf16, tag="mrow", name=f"mrow_{b}")
            nc.scalar.dma_start(
                mrow[:], msk.ap()[m_off : m_off + rows].rearrange("(o r) -> o r", o=1)
            )

            rh_list = [
                hpool.tile([128, rows], f32r, tag=f"rh{j}", name=f"rh{j}_{b}")
                for j in range(HC)
            ]
            erow = spool.tile([1, rows], bf16, tag="erow", name=f"erow_{b}")
            if rows > 512:
                c0 = ((rows // 2) + 1) // 2 * 2   # even split point
                chunks = [(0, c0), (c0, rows - c0)]
            else:
                chunks = [(0, rows)]
            for (coff, csz) in chunks:
                cs = slice(coff, coff + csz)
                for j in range(HC):
                    ph = ph_pool.tile(
                        [128, csz], f32, tag="h", name=f"ph{j}_{b}_{coff}"
                    )
                    for k in range(KC):
                        nc.tensor.matmul(
                            ph[:],
                            t_w1[:, k, 128 * j : 128 * (j + 1)],
                            x_r[:, k, cs],
                            start=(k == 0),
                            stop=(k == KC - 1),
                        )
                    nc.scalar.activation(
                        rh_list[j][:, cs], ph[:], AF.Relu, bias=t_b1[:, j : j + 1]
                    )

                # scores + additive length mask, one PSUM accumulation group
                ps_s = ps_pool.tile([1, csz], f32, tag="s", name=f"ps_{b}_{coff}")
                nc.tensor.matmul(
                    ps_s[:], t_one1[:], mrow[:, cs], start=True, stop=False,
                    skip_group_check=True,
                )
                for j in range(HC):
                    nc.tensor.matmul(
                        ps_s[:], t_w2[:, j : j + 1], rh_list[j][:, cs],
                        start=False, stop=(j == HC - 1),
                        skip_group_check=True,
                    )
                nc.scalar.activation(erow[:, cs], ps_s[:], AF.Exp)

            wful = wpool.tile([128, rows], bf16, tag="wf", name=f"wf_{b}")
            nc.gpsimd.partition_broadcast(wful[:], erow[:])

            wsum = spool.tile([128, bp], f32, tag="wsum", name=f"wsum_{b}")
            nc.vector.reduce_sum(
                wsum[:], wful[:].rearrange("p (s l) -> p s l", l=cap), axis=AX.X
            )
            winv = spool.tile([128, bp], f32, tag="winv", name=f"winv_{b}")
            nc.vector.reciprocal(winv[:], wsum[:])

            xw = xwpool.tile([128, KC, rows], bf16, tag="xw", name=f"xw_{b}")
            for k in range(KC):
                eng = nc.gpsimd if k == KC - 1 else nc.vector
                eng.tensor_mul(xw[:, k, :], x_b[:, k, :], wful[:])
            praw = spool.tile([128, KC, bp], f32, tag="praw", name=f"praw_{b}")
            nc.vector.reduce_sum(
                praw[:],
                xw[:].rearrange("p k (s l) -> p k s l", l=cap),
                axis=AX.X,
            )
            winv_bc = winv[:].rearrange("p (x s) -> p x s", x=1).to_broadcast(
                [128, KC, bp]
            )
            nc.vector.tensor_mul(
                pfT[:, :, p_off : p_off + bp], praw[:], winv_bc
            )

            if ei == 0:
                nc.sync.dma_start(t_aw1[:], aw1.ap().rearrange("k d h -> d k h"))
                nc.sync.dma_start(t_ab1[:], ab1.ap())
                nc.sync.dma_start(t_aw2[:], aw2.ap())

        if stage == "wsum":
            nc.sync.dma_start(dbg.ap(), pfT[:].bitcast(f32))

        if stage == "full":
            # ---- stage 2: path-level attention (f32r matmuls) ----
            pfr = pfT[:]
            rh2_list = []
            for j in range(HC):
                ph2 = ph_pool.tile([128, PS], f32, tag="h")
                for k in range(KC):
                    nc.tensor.matmul(
                        ph2[:],
                        t_aw1[:, k, 128 * j : 128 * (j + 1)],
                        pfr[:, k, :],
                        start=(k == 0),
                        stop=(k == KC - 1),
                    )
                rh2 = hpool.tile([128, PS], f32r, tag=f"rh2{j}")
                nc.scalar.activation(rh2[:], ph2[:], AF.Relu, bias=t_ab1[:, j : j + 1])
                rh2_list.append(rh2)

            ps_a = ps_pool.tile([1, PS], f32, tag="s")
            for j in range(HC):
                nc.tensor.matmul(
                    ps_a[:], t_aw2[:, j : j + 1], rh2_list[j][:],
                    start=(j == 0), stop=(j == HC - 1),
                )

            negm = spool.tile([1, 1], f32, tag="negm")
            nc.vector.reduce_max(negm[:], ps_a[:], axis=AX.X, negate=True)
            ea = spool.tile([1, PS], f32, tag="ea")
            s_t = spool.tile([1, 1], f32, tag="s1")
            nc.scalar.activation(ea[:], ps_a[:], AF.Exp, bias=negm[:], accum_out=s_t[:])

            ebc = wpool.tile([128, PS], f32, tag="ebc")
            nc.gpsimd.partition_broadcast(ebc[:], ea[:])

            part = spool.tile([128, KC], f32, tag="part")
            for k in range(KC):
                scr = wpool.tile([128, PS], f32, tag="scr", name=f"scr_{k}")
                nc.vector.tensor_mul(scr[:], pfT[:, k, :].bitcast(f32), ebc[:])
                nc.vector.reduce_sum(part[:, k : k + 1], scr[:], axis=AX.X)
            nc.sync.dma_start(out_part.ap(), part[:])
            nc.sync.dma_start(out_stats.ap()[:, 0:1], negm[:])
            nc.sync.dma_start(out_stats.ap()[:, 1:2], s_t[:])

    nc.compile()
    return nc


def _get_program(blocks, stage="full"):
    key = (blocks, stage)
    if key not in _PROG_CACHE:
        _PROG_CACHE[key] = _build_program(blocks, stage)
    return _PROG_CACHE[key]


def _prep(inputs):
    """Host-side sharding/sorting/packing. Returns (blocks, in_maps)."""
    x = np.asarray(inputs["paths_nodes"], dtype=np.float32)
    lengths = np.asarray(inputs["lengths"], dtype=np.int32)
    pW1 = np.asarray(inputs["pW1"], dtype=np.float32)
    pb1 = np.asarray(inputs["pb1"], dtype=np.float32)
    pw2 = np.asarray(inputs["pw2"], dtype=np.float32)
    aW1 = np.asarray(inputs["aW1"], dtype=np.float32)
    ab1 = np.asarray(inputs["ab1"], dtype=np.float32)
    aw2 = np.asarray(inputs["aw2"], dtype=np.float32)
    # pb2 / ab2 shift their softmax logits uniformly -> no effect on output.

    bf = ml_dtypes.bfloat16
    len_sh = lengths.reshape(NCORES, PS)
    orders = np.argsort(-len_sh, axis=1, kind="stable")        # [NC, PS] desc
    sorted_len = np.take_along_axis(len_sh, orders, axis=1)
    len_max = sorted_len.max(axis=0)                           # [PS]
    blocks = _make_blocks(len_max)

    x_sh = x.reshape(NCORES, PS, LMAX, D)
    w1_np = np.ascontiguousarray(pW1.reshape(KC, 128, H)).astype(np.float32)
    w2_np = np.ascontiguousarray(pw2.reshape(HC, 128).T).astype(np.float32)
    b1_np = np.ascontiguousarray(pb1.reshape(HC, 128).T).astype(np.float32)
    aw1_np = np.ascontiguousarray(aW1.reshape(KC, 128, H)).astype(np.float32)
    ab1_np = np.ascontiguousarray(ab1.reshape(HC, 128).T).astype(np.float32)
    aw2_np = np.ascontiguousarray(aw2.reshape(HC, 128).T).astype(np.float32)
    one1 = np.ones((1, 1), dtype=bf)

    ar = np.arange(LMAX)
    in_maps = []
    for c in range(NCORES):
        xc = x_sh[c][orders[c]]                       # [PS, LMAX, D] sorted
        lc = sorted_len[c]                            # [PS]
        xr_parts = []
        mk_parts = []
        p = 0
        for (bp, cap) in blocks:
            xblk = xc[p : p + bp, :cap, :]            # [bp, cap, D]
            xb_t = (
                xblk.reshape(bp, cap, KC, 128)
                .transpose(2, 3, 0, 1)
                .reshape(KC, 128, bp * cap)
            )
            xr_parts.append(xb_t.astype(bf).ravel())
            lb = lc[p : p + bp]
            mk = np.where(ar[None, :cap] < lb[:, None], 0.0, MASK_NEG)
            mk_parts.append(mk.astype(bf).ravel())
            p += bp
        in_maps.append(
            {
                "xb": np.concatenate(xr_parts),
                "msk": np.concatenate(mk_parts),
                "w1": w1_np,
                "w2": w2_np,
                "b1": b1_np,
                "aw1": aw1_np,
                "ab1": ab1_np,
                "aw2": aw2_np,
                "one1_bf": one1,
            }
        )
    return blocks, in_maps


def kernel(**inputs):
    global LAST_RESULT
    blocks, in_maps = _prep(inputs)
    nc = _get_program(blocks)

    res = bass_utils.run_bass_kernel_spmd(
        nc, in_maps, core_ids=list(range(NCORES)), **_TRACE_KW
    )
    LAST_RESULT = res

    parts = np.stack([r["out_part"] for r in res.results])    # [8, 128, KC]
    stats = np.stack([r["out_stats"] for r in res.results])   # [8, 1, 2]
    m = -stats[:, 0, 0]
    s = stats[:, 0, 1]
    mg = m.max()
    sc = np.exp(m - mg)
    total = float((sc * s).sum())
    vec = (parts * sc[:, None, None]).sum(axis=0)             # [128, KC]
    user = np.ascontiguousarray(vec.T).reshape(D) / total
    return user.astype(np.float32)


# revision 29
# speedup vs baseline: 1.6723x; 1.6723x over previous
"""Trainium2 Bass kernel for nn_AttentionNetwork (ragged path attention).

Data-parallel over 8 NeuronCores: 512 paths per core. Paths are sorted by
length (host-side) and packed into variable-width blocks (bp paths x cap
node-slots, bp*cap <= 1024, cap = max length in the block; capacities are
taken as the element-wise max over cores so one SPMD program serves all
8). This skips the ~45% of node slots beyond each path's length that a
fixed 64-slot layout would waste. Per block: node-MLP in float32r
(TF32-like PE mode: full bf16 throughput, ~1.5e-4 matmul error),
length-masked softmax over nodes (additive mask folded into the
score-matmul PSUM group as a K=1 accumulate; exp row broadcast across
partitions on GpSimd), then the softmax-weighted node sum on the vector
engine from a bf16 copy of X. Stage 2 (f32r) computes path-attention
scores and returns exp-weighted partial sums + (max, sumexp) stats; the
host combines the 8 partials (softmax over paths is permutation-
invariant, so the sorted order needs no undoing).
"""

import sys

if "/opt/trn_rl_repo" not in sys.path:
    sys.path.insert(0, "/opt/trn_rl_repo")

from contextlib import ExitStack

import ml_dtypes
import numpy as np

import concourse.bass as bass  # noqa: F401
import concourse.mybir as mybir
import concourse.tile as tile
from concourse import bacc, bass_utils

P, LMAX, D, H = 4096, 64, 512, 512
NCORES = 8
PS = P // NCORES          # paths per core
KC = D // 128             # contraction chunks
HC = H // 128             # hidden tiles
MASK_NEG = -30000.0
ROWS_TARGET = 1024        # max rows (bp*cap) per block

f32 = mybir.dt.float32
f32r = mybir.dt.float32r
bf16 = mybir.dt.bfloat16
AF = mybir.ActivationFunctionType
ALU = mybir.AluOpType
AX = mybir.AxisListType

LAST_RESULT = None
_PROG_CACHE = {}
_TRACE_KW = {}


def _make_blocks(len_max):
    """Greedy pack sorted-desc lengths into (bp, cap) blocks, bp*cap<=1024."""
    blocks = []
    i = 0
    while i < PS:
        cap = int(len_max[i])
        bp = min(ROWS_TARGET // cap, PS - i)
        if (bp * cap) % 2:
            cap += 1          # keep matmul free dims even (fp32r ISA rule)
        blocks.append((bp, cap))
        i += bp
    return tuple(blocks)


def _build_program(blocks, stage="full"):
    """blocks: tuple of (bp, cap); one block = bp paths x cap node slots."""
    nb = len(blocks)
    rows_list = [bp * cap for bp, cap in blocks]
    tot_rows = sum(rows_list)

    nc = bacc.Bacc("TRN2", target_bir_lowering=False, debug=False, num_devices=NCORES)

    xb = nc.dram_tensor("xb", [KC * 128 * tot_rows], bf16, kind="ExternalInput")
    msk = nc.dram_tensor("msk", [tot_rows], bf16, kind="ExternalInput")
    w1 = nc.dram_tensor("w1", [KC, 128, H], bf16, kind="ExternalInput")
    w2 = nc.dram_tensor("w2", [128, HC], f32r, kind="ExternalInput")
    b1 = nc.dram_tensor("b1", [128, HC], f32, kind="ExternalInput")
    aw1 = nc.dram_tensor("aw1", [KC, 128, H], f32r, kind="ExternalInput")
    ab1 = nc.dram_tensor("ab1", [128, HC], f32, kind="ExternalInput")
    aw2 = nc.dram_tensor("aw2", [128, HC], f32r, kind="ExternalInput")
    one1_bf = nc.dram_tensor("one1_bf", [1, 1], bf16, kind="ExternalInput")
    out_part = nc.dram_tensor("out_part", [128, KC], f32, kind="ExternalOutput")
    out_stats = nc.dram_tensor("out_stats", [1, 2], f32, kind="ExternalOutput")
    dbg = None
    if stage != "full":
        dbg = nc.dram_tensor("dbg", [128, KC, PS], f32, kind="ExternalOutput")

    with ExitStack() as ctx:
        tc = ctx.enter_context(tile.TileContext(nc))
        const = ctx.enter_context(tc.tile_pool(name="const", bufs=1))
        xpool = ctx.enter_context(tc.tile_pool(name="x", bufs=3))
        xwpool = ctx.enter_context(tc.tile_pool(name="xw", bufs=2))
        hpool = ctx.enter_context(tc.tile_pool(name="h", bufs=2))
        wpool = ctx.enter_context(tc.tile_pool(name="w", bufs=2))
        spool = ctx.enter_context(tc.tile_pool(name="s", bufs=3))
        ph_pool = ctx.enter_context(tc.tile_pool(name="ph", bufs=6, space="PSUM"))
        ps_pool = ctx.enter_context(tc.tile_pool(name="ps", bufs=2, space="PSUM"))

        t_w1b = const.tile([128, KC, H], bf16)
        nc.sync.dma_start(t_w1b[:], w1.ap().rearrange("k d h -> d k h"))
        t_w1 = const.tile([128, KC, H], f32r)
        nc.scalar.copy(t_w1[:], t_w1b[:])
        t_w2 = const.tile([128, HC], f32r)
        nc.sync.dma_start(t_w2[:], w2.ap())
        t_b1 = const.tile([128, HC], f32)
        nc.sync.dma_start(t_b1[:], b1.ap())
        t_one1 = const.tile([1, 1], bf16)
        nc.sync.dma_start(t_one1[:], one1_bf.ap())
        # ACT table prefetch: force the exp_and_others load before data arrives
        t_warm = const.tile([1, 1], f32)
        nc.scalar.activation(t_warm[:], t_one1[:], AF.Exp)
        t_aw1 = const.tile([128, KC, H], f32r)
        t_ab1 = const.tile([128, HC], f32)
        t_aw2 = const.tile([128, HC], f32r)

        pfT = const.tile([128, KC, PS], f32r)  # normalized path features

        x_offs = [0] * nb
        m_offs = [0] * nb
        p_offs = [0] * nb
        acc_x = acc_m = acc_p = 0
        for i in range(nb):
            x_offs[i], m_offs[i], p_offs[i] = acc_x, acc_m, acc_p
            acc_x += KC * 128 * rows_list[i]
            acc_m += rows_list[i]
            acc_p += blocks[i][0]
        assert acc_p == PS

        emit_order = [nb - 1] + list(range(nb - 1))
        for ei, b in enumerate(emit_order):
            bp, cap = blocks[b]
            rows = rows_list[b]
            x_off, m_off, p_off = x_offs[b], m_offs[b], p_offs[b]

            x_b = xpool.tile([128, KC, rows], bf16, tag="xb", name=f"xb_{b}")
            nc.sync.dma_start(
                x_b[:],
                xb.ap()[x_off : x_off + KC * 128 * rows].rearrange(
                    "(k d r) -> d k r", k=KC, d=128
                ),
            )
            x_r = xpool.tile([128, KC, rows], f32r, tag="xr", name=f"xr_{b}")
            nc.scalar.copy(x_r[:, 0:2, :], x_b[:, 0:2, :])
            nc.vector.tensor_copy(x_r[:, 2:4, :], x_b[:, 2:4, :])
            mrow = spool.tile([1, rows], bf16, tag="mrow", name=f"mrow_{b}")
            nc.scalar.dma_start(
                mrow[:], msk.ap()[m_off : m_off + rows].rearrange("(o r) -> o r", o=1)
            )

            rh_list = [
                hpool.tile([128, rows], f32r, tag=f"rh{j}", name=f"rh{j}_{b}")
                for j in range(HC)
            ]
            erow = spool.tile([1, rows], bf16, tag="erow", name=f"erow_{b}")
            if rows > 512:
                c0 = ((rows // 2) + 1) // 2 * 2   # even split point
                chunks = [(0, c0), (c0, rows - c0)]
            else:
                chunks = [(0, rows)]
            for (coff, csz) in chunks:
                cs = slice(coff, coff + csz)
                for j in range(HC):
                    ph = ph_pool.tile(
                        [128, csz], f32, tag="h", name=f"ph{j}_{b}_{coff}"
                    )
                    for k in range(KC):
                        nc.tensor.matmul(
                            ph[:],
                            t_w1[:, k, 128 * j : 128 * (j + 1)],
                            x_r[:, k, cs],
                            start=(k == 0),
                            stop=(k == KC - 1),
                        )
                    nc.scalar.activation(
                        rh_list[j][:, cs], ph[:], AF.Relu, bias=t_b1[:, j : j + 1]
                    )

                # scores + additive length mask, one PSUM accumulation group
                ps_s = ps_pool.tile([1, csz], f32, tag="s", name=f"ps_{b}_{coff}")
                nc.tensor.matmul(
                    ps_s[:], t_one1[:], mrow[:, cs], start=True, stop=False,
                    skip_group_check=True,
                )
                for j in range(HC):
                    nc.tensor.matmul(
                        ps_s[:], t_w2[:, j : j + 1], rh_list[j][:, cs],
                        start=False, stop=(j == HC - 1),
                        skip_group_check=True,
                    )
                nc.scalar.activation(erow[:, cs], ps_s[:], AF.Exp)

            wful = wpool.tile([128, rows], bf16, tag="wf", name=f"wf_{b}")
            nc.gpsimd.partition_broadcast(wful[:], erow[:])

            wsum = spool.tile([128, bp], f32, tag="wsum", name=f"wsum_{b}")
            nc.vector.reduce_sum(
                wsum[:], wful[:].rearrange("p (s l) -> p s l", l=cap), axis=AX.X
            )
            winv = spool.tile([128, bp], f32, tag="winv", name=f"winv_{b}")
            nc.vector.reciprocal(winv[:], wsum[:])

            xw = xwpool.tile([128, KC, rows], bf16, tag="xw", name=f"xw_{b}")
            for k in range(KC):
                nc.vector.tensor_mul(xw[:, k, :], x_b[:, k, :], wful[:])
            praw = spool.tile([128, KC, bp], f32, tag="praw", name=f"praw_{b}")
            nc.vector.reduce_sum(
                praw[:],
                xw[:].rearrange("p k (s l) -> p k s l", l=cap),
                axis=AX.X,
            )
            winv_bc = winv[:].rearrange("p (x s) -> p x s", x=1).to_broadcast(
                [128, KC, bp]
            )
            nc.vector.tensor_mul(
                pfT[:, :, p_off : p_off + bp], praw[:], winv_bc
            )

            if ei == 0:
                nc.sync.dma_start(t_aw1[:], aw1.ap().rearrange("k d h -> d k h"))
                nc.sync.dma_start(t_ab1[:], ab1.ap())
                nc.sync.dma_start(t_aw2[:], aw2.ap())

        if stage == "wsum":
            nc.sync.dma_start(dbg.ap(), pfT[:].bitcast(f32))

        if stage == "full":
            # ---- stage 2: path-level attention (f32r matmuls) ----
            pfr = pfT[:]
            rh2_list = []
            for j in range(HC):
                ph2 = ph_pool.tile([128, PS], f32, tag="h")
                for k in range(KC):
                    nc.tensor.matmul(
                        ph2[:],
                        t_aw1[:, k, 128 * j : 128 * (j + 1)],
                        pfr[:, k, :],
                        start=(k == 0),
                        stop=(k == KC - 1),
                    )
                rh2 = hpool.tile([128, PS], f32r, tag=f"rh2{j}")
                nc.scalar.activation(rh2[:], ph2[:], AF.Relu, bias=t_ab1[:, j : j + 1])
                rh2_list.append(rh2)

            ps_a = ps_pool.tile([1, PS], f32, tag="s")
            for j in range(HC):
                nc.tensor.matmul(
                    ps_a[:], t_aw2[:, j : j + 1], rh2_list[j][:],
                    start=(j == 0), stop=(j == HC - 1),
                )

            negm = spool.tile([1, 1], f32, tag="negm")
            nc.vector.reduce_max(negm[:], ps_a[:], axis=AX.X, negate=True)
            ea = spool.tile([1, PS], f32, tag="ea")
            s_t = spool.tile([1, 1], f32, tag="s1")
            nc.scalar.activation(ea[:], ps_a[:], AF.Exp, bias=negm[:], accum_out=s_t[:])

            ebc = wpool.tile([128, PS], f32, tag="ebc")
            nc.gpsimd.partition_broadcast(ebc[:], ea[:])

            part = spool.tile([128, KC], f32, tag="part")
            for k in range(KC):
                scr = wpool.tile([128, PS], f32, tag="scr", name=f"scr_{k}")
                nc.vector.tensor_mul(scr[:], pfT[:, k, :].bitcast(f32), ebc[:])
                nc.vector.reduce_sum(part[:, k : k + 1], scr[:], axis=AX.X)
            nc.sync.dma_start(out_part.ap(), part[:])
            nc.sync.dma_start(out_stats.ap()[:, 0:1], negm[:])
            nc.sync.dma_start(out_stats.ap()[:, 1:2], s_t[:])

    nc.compile()
    return nc


def _get_program(blocks, stage="full"):
    key = (blocks, stage)
    if key not in _PROG_CACHE:
        _PROG_CACHE[key] = _build_program(blocks, stage)
    return _PROG_CACHE[key]


def _prep(inputs):
    """Host-side sharding/sorting/packing. Returns (blocks, in_maps)."""
    x = np.asarray(inputs["paths_nodes"], dtype=np.float32)
    lengths = np.asarray(inputs["lengths"], dtype=np.int32)
    pW1 = np.asarray(inputs["pW1"], dtype=np.float32)
    pb1 = np.asarray(inputs["pb1"], dtype=np.float32)
    pw2 = np.asarray(inputs["pw2"], dtype=np.float32)
    aW1 = np.asarray(inputs["aW1"], dtype=np.float32)
    ab1 = np.asarray(inputs["ab1"], dtype=np.float32)
    aw2 = np.asarray(inputs["aw2"], dtype=np.float32)
    # pb2 / ab2 shift their softmax logits uniformly -> no effect on output.

    bf = ml_dtypes.bfloat16
    len_sh = lengths.reshape(NCORES, PS)
    orders = np.argsort(-len_sh, axis=1, kind="stable")        # [NC, PS] desc
    sorted_len = np.take_along_axis(len_sh, orders, axis=1)
    len_max = sorted_len.max(axis=0)                           # [PS]
    blocks = _make_blocks(len_max)

    x_sh = x.reshape(NCORES, PS, LMAX, D)
    w1_np = np.ascontiguousarray(pW1.reshape(KC, 128, H)).astype(bf)
    w2_np = np.ascontiguousarray(pw2.reshape(HC, 128).T).astype(np.float32)
    b1_np = np.ascontiguousarray(pb1.reshape(HC, 128).T).astype(np.float32)
    aw1_np = np.ascontiguousarray(aW1.reshape(KC, 128, H)).astype(np.float32)
    ab1_np = np.ascontiguousarray(ab1.reshape(HC, 128).T).astype(np.float32)
    aw2_np = np.ascontiguousarray(aw2.reshape(HC, 128).T).astype(np.float32)
    one1 = np.ones((1, 1), dtype=bf)

    ar = np.arange(LMAX)
    in_maps = []
    for c in range(NCORES):
        xc = x_sh[c][orders[c]]                       # [PS, LMAX, D] sorted
        lc = sorted_len[c]                            # [PS]
        xr_parts = []
        mk_parts = []
        p = 0
        for (bp, cap) in blocks:
            xblk = xc[p : p + bp, :cap, :]            # [bp, cap, D]
            xb_t = (
                xblk.reshape(bp, cap, KC, 128)
                .transpose(2, 3, 0, 1)
                .reshape(KC, 128, bp * cap)
            )
            xr_parts.append(xb_t.astype(bf).ravel())
            lb = lc[p : p + bp]
            mk = np.where(ar[None, :cap] < lb[:, None], 0.0, MASK_NEG)
            mk_parts.append(mk.astype(bf).ravel())
            p += bp
        in_maps.append(
            {
                "xb": np.concatenate(xr_parts),
                "msk": np.concatenate(mk_parts),
                "w1": w1_np,
                "w2": w2_np,
                "b1": b1_np,
                "aw1": aw1_np,
                "ab1": ab1_np,
                "aw2": aw2_np,
                "one1_bf": one1,
            }
        )
    return blocks, in_maps


def kernel(**inputs):
    global LAST_RESULT
    blocks, in_maps = _prep(inputs)
    nc = _get_program(blocks)

    res = bass_utils.run_bass_kernel_spmd(
        nc, in_maps, core_ids=list(range(NCORES)), **_TRACE_KW
    )
    LAST_RESULT = res

    parts = np.stack([r["out_part"] for r in res.results])    # [8, 128, KC]
    stats = np.stack([r["out_stats"] for r in res.results])   # [8, 1, 2]
    m = -stats[:, 0, 0]
    s = stats[:, 0, 1]
    mg = m.max()
    sc = np.exp(m - mg)
    total = float((sc * s).sum())
    vec = (parts * sc[:, None, None]).sum(axis=0)             # [128, KC]
    user = np.ascontiguousarray(vec.T).reshape(D) / total
    return user.astype(np.float32)


# revision 32
# speedup vs baseline: 1.6831x; 1.0064x over previous
"""Trainium2 Bass kernel for nn_AttentionNetwork (ragged path attention).

Data-parallel over 8 NeuronCores: 512 paths per core. Paths are sorted by
length (host-side) and packed into variable-width blocks (bp paths x cap
node-slots, bp*cap <= 1024, cap = max length in the block; capacities are
taken as the element-wise max over cores so one SPMD program serves all
8). This skips the ~45% of node slots beyond each path's length that a
fixed 64-slot layout would waste. Per block: node-MLP in float32r
(TF32-like PE mode: full bf16 throughput, ~1.5e-4 matmul error),
length-masked softmax over nodes (additive mask folded into the
score-matmul PSUM group as a K=1 accumulate; exp row broadcast across
partitions on GpSimd), then the softmax-weighted node sum on the vector
engine from a bf16 copy of X. Stage 2 (f32r) computes path-attention
scores and returns exp-weighted partial sums + (max, sumexp) stats; the
host combines the 8 partials (softmax over paths is permutation-
invariant, so the sorted order needs no undoing).
"""

import sys

if "/opt/trn_rl_repo" not in sys.path:
    sys.path.insert(0, "/opt/trn_rl_repo")

from contextlib import ExitStack

import ml_dtypes
import numpy as np

import concourse.bass as bass  # noqa: F401
import concourse.mybir as mybir
import concourse.tile as tile
from concourse import bacc, bass_utils

P, LMAX, D, H = 4096, 64, 512, 512
NCORES = 8
PS = P // NCORES          # paths per core
KC = D // 128             # contraction chunks
HC = H // 128             # hidden tiles
MASK_NEG = -30000.0
ROWS_TARGET = 1024        # max rows (bp*cap) per block

f32 = mybir.dt.float32
f32r = mybir.dt.float32r
bf16 = mybir.dt.bfloat16
AF = mybir.ActivationFunctionType
ALU = mybir.AluOpType
AX = mybir.AxisListType

LAST_RESULT = None
_PROG_CACHE = {}
_TRACE_KW = {}


def _make_blocks(len_max):
    """Greedy pack sorted-desc lengths into (bp, cap) blocks, bp*cap<=1024."""
    blocks = []
    i = 0
    while i < PS:
        cap = int(len_max[i])
        bp = min(ROWS_TARGET // cap, PS - i)
        if (bp * cap) % 2:
            cap += 1          # keep matmul free dims even (fp32r ISA rule)
        blocks.append((bp, cap))
        i += bp
    return tuple(blocks)


def _build_program(blocks, stage="full"):
    """blocks: tuple of (bp, cap); one block = bp paths x cap node slots."""
    nb = len(blocks)
    rows_list = [bp * cap for bp, cap in blocks]
    tot_rows = sum(rows_list)

    nc = bacc.Bacc("TRN2", target_bir_lowering=False, debug=False, num_devices=NCORES)

    xb = nc.dram_tensor("xb", [KC * 128 * tot_rows], bf16, kind="ExternalInput")
    msk = nc.dram_tensor("msk", [tot_rows], bf16, kind="ExternalInput")
    w1 = nc.dram_tensor("w1", [KC, 128, H], f32r, kind="ExternalInput")
    w2 = nc.dram_tensor("w2", [128, HC], f32r, kind="ExternalInput")
    b1 = nc.dram_tensor("b1", [128, HC], f32, kind="ExternalInput")
    aw1 = nc.dram_tensor("aw1", [KC, 128, H], f32r, kind="ExternalInput")
    ab1 = nc.dram_tensor("ab1", [128, HC], f32, kind="ExternalInput")
    aw2 = nc.dram_tensor("aw2", [128, HC], f32r, kind="ExternalInput")
    one1_bf = nc.dram_tensor("one1_bf", [1, 1], bf16, kind="ExternalInput")
    out_part = nc.dram_tensor("out_part", [128, KC], f32, kind="ExternalOutput")
    out_stats = nc.dram_tensor("out_stats", [1, 2], f32, kind="ExternalOutput")
    dbg = None
    if stage != "full":
        dbg = nc.dram_tensor("dbg", [128, KC, PS], f32, kind="ExternalOutput")

    with ExitStack() as ctx:
        tc = ctx.enter_context(tile.TileContext(nc))
        const = ctx.enter_context(tc.tile_pool(name="const", bufs=1))
        xpool = ctx.enter_context(tc.tile_pool(name="x", bufs=3))
        xwpool = ctx.enter_context(tc.tile_pool(name="xw", bufs=2))
        hpool = ctx.enter_context(tc.tile_pool(name="h", bufs=2))
        wpool = ctx.enter_context(tc.tile_pool(name="w", bufs=2))
        spool = ctx.enter_context(tc.tile_pool(name="s", bufs=3))
        ph_pool = ctx.enter_context(tc.tile_pool(name="ph", bufs=6, space="PSUM"))
        ps_pool = ctx.enter_context(tc.tile_pool(name="ps", bufs=2, space="PSUM"))

        t_w1 = const.tile([128, KC, H], f32r)
        nc.sync.dma_start(t_w1[:], w1.ap().rearrange("k d h -> d k h"))
        t_w2 = const.tile([128, HC], f32r)
        nc.sync.dma_start(t_w2[:], w2.ap())
        t_b1 = const.tile([128, HC], f32)
        nc.sync.dma_start(t_b1[:], b1.ap())
        t_one1 = const.tile([1, 1], bf16)
        nc.sync.dma_start(t_one1[:], one1_bf.ap())
        # ACT table prefetch: force the exp_and_others load before data arrives
        t_warm = const.tile([1, 1], f32)
        nc.scalar.activation(t_warm[:], t_one1[:], AF.Exp)
        t_aw1 = const.tile([128, KC, H], f32r)
        t_ab1 = const.tile([128, HC], f32)
        t_aw2 = const.tile([128, HC], f32r)

        pfT = const.tile([128, KC, PS], f32r)  # normalized path features

        x_offs = [0] * nb
        m_offs = [0] * nb
        p_offs = [0] * nb
        acc_x = acc_m = acc_p = 0
        for i in range(nb):
            x_offs[i], m_offs[i], p_offs[i] = acc_x, acc_m, acc_p
            acc_x += KC * 128 * rows_list[i]
            acc_m += rows_list[i]
            acc_p += blocks[i][0]
        assert acc_p == PS

        emit_order = [nb - 1] + list(range(nb - 1))
        for ei, b in enumerate(emit_order):
            bp, cap = blocks[b]
            rows = rows_list[b]
            x_off, m_off, p_off = x_offs[b], m_offs[b], p_offs[b]

            x_b = xpool.tile([128, KC, rows], bf16, tag="xb", name=f"xb_{b}", bufs=4)
            nc.sync.dma_start(
                x_b[:],
                xb.ap()[x_off : x_off + KC * 128 * rows].rearrange(
                    "(k d r) -> d k r", k=KC, d=128
                ),
            )
            x_r = xpool.tile([128, KC, rows], f32r, tag="xr", name=f"xr_{b}")
            nc.scalar.copy(x_r[:, 0:2, :], x_b[:, 0:2, :])
            nc.vector.tensor_copy(x_r[:, 2:4, :], x_b[:, 2:4, :])
            mrow = spool.tile([1, rows], bf16, tag="mrow", name=f"mrow_{b}")
            nc.scalar.dma_start(
                mrow[:], msk.ap()[m_off : m_off + rows].rearrange("(o r) -> o r", o=1)
            )

            rh_list = [
                hpool.tile([128, rows], f32r, tag=f"rh{j}", name=f"rh{j}_{b}")
                for j in range(HC)
            ]
            erow = spool.tile([1, rows], bf16, tag="erow", name=f"erow_{b}")
            if rows > 512:
                c0 = ((rows // 2) + 1) // 2 * 2   # even split point
                chunks = [(0, c0), (c0, rows - c0)]
            else:
                chunks = [(0, rows)]
            for (coff, csz) in chunks:
                cs = slice(coff, coff + csz)
                for j in range(HC):
                    ph = ph_pool.tile(
                        [128, csz], f32, tag="h", name=f"ph{j}_{b}_{coff}"
                    )
                    for k in range(KC):
                        nc.tensor.matmul(
                            ph[:],
                            t_w1[:, k, 128 * j : 128 * (j + 1)],
                            x_r[:, k, cs],
                            start=(k == 0),
                            stop=(k == KC - 1),
                        )
                    nc.scalar.activation(
                        rh_list[j][:, cs], ph[:], AF.Relu, bias=t_b1[:, j : j + 1]
                    )

                # scores + additive length mask, one PSUM accumulation group
                ps_s = ps_pool.tile([1, csz], f32, tag="s", name=f"ps_{b}_{coff}")
                nc.tensor.matmul(
                    ps_s[:], t_one1[:], mrow[:, cs], start=True, stop=False,
                    skip_group_check=True,
                )
                for j in range(HC):
                    nc.tensor.matmul(
                        ps_s[:], t_w2[:, j : j + 1], rh_list[j][:, cs],
                        start=False, stop=(j == HC - 1),
                        skip_group_check=True,
                    )
                nc.scalar.activation(erow[:, cs], ps_s[:], AF.Exp)

            wful = wpool.tile([128, rows], bf16, tag="wf", name=f"wf_{b}", bufs=3)
            nc.gpsimd.partition_broadcast(wful[:], erow[:])

            wsum = spool.tile([128, bp], f32, tag="wsum", name=f"wsum_{b}")
            nc.vector.reduce_sum(
                wsum[:], wful[:].rearrange("p (s l) -> p s l", l=cap), axis=AX.X
            )
            winv = spool.tile([128, bp], f32, tag="winv", name=f"winv_{b}")
            nc.vector.reciprocal(winv[:], wsum[:])

            xw = xwpool.tile([128, KC, rows], bf16, tag="xw", name=f"xw_{b}")
            for k in range(KC):
                nc.vector.tensor_mul(xw[:, k, :], x_b[:, k, :], wful[:])
            praw = spool.tile([128, KC, bp], f32, tag="praw", name=f"praw_{b}")
            nc.vector.reduce_sum(
                praw[:],
                xw[:].rearrange("p k (s l) -> p k s l", l=cap),
                axis=AX.X,
            )
            winv_bc = winv[:].rearrange("p (x s) -> p x s", x=1).to_broadcast(
                [128, KC, bp]
            )
            nc.vector.tensor_mul(
                pfT[:, :, p_off : p_off + bp], praw[:], winv_bc
            )

            if ei == 0:
                nc.sync.dma_start(t_aw1[:], aw1.ap().rearrange("k d h -> d k h"))
                nc.sync.dma_start(t_ab1[:], ab1.ap())
                nc.sync.dma_start(t_aw2[:], aw2.ap())

        if stage == "wsum":
            nc.sync.dma_start(dbg.ap(), pfT[:].bitcast(f32))

        if stage == "full":
            # ---- stage 2: path-level attention (f32r matmuls) ----
            pfr = pfT[:]
            rh2_list = []
            for j in range(HC):
                ph2 = ph_pool.tile([128, PS], f32, tag="h")
                for k in range(KC):
                    nc.tensor.matmul(
                        ph2[:],
                        t_aw1[:, k, 128 * j : 128 * (j + 1)],
                        pfr[:, k, :],
                        start=(k == 0),
                        stop=(k == KC - 1),
                    )
                rh2 = hpool.tile([128, PS], f32r, tag=f"rh2{j}")
                nc.scalar.activation(rh2[:], ph2[:], AF.Relu, bias=t_ab1[:, j : j + 1])
                rh2_list.append(rh2)

            ps_a = ps_pool.tile([1, PS], f32, tag="s")
            for j in range(HC):
                nc.tensor.matmul(
                    ps_a[:], t_aw2[:, j : j + 1], rh2_list[j][:],
                    start=(j == 0), stop=(j == HC - 1),
                )

            negm = spool.tile([1, 1], f32, tag="negm")
            nc.vector.reduce_max(negm[:], ps_a[:], axis=AX.X, negate=True)
            ea = spool.tile([1, PS], f32, tag="ea")
            s_t = spool.tile([1, 1], f32, tag="s1")
            nc.scalar.activation(ea[:], ps_a[:], AF.Exp, bias=negm[:], accum_out=s_t[:])

            ebc = wpool.tile([128, PS], f32, tag="ebc")
            nc.gpsimd.partition_broadcast(ebc[:], ea[:])

            part = spool.tile([128, KC], f32, tag="part")
            for k in range(KC):
                scr = wpool.tile([128, PS], f32, tag="scr", name=f"scr_{k}")
                nc.vector.tensor_mul(scr[:], pfT[:, k, :].bitcast(f32), ebc[:])
                nc.vector.reduce_sum(part[:, k : k + 1], scr[:], axis=AX.X)
            nc.sync.dma_start(out_part.ap(), part[:])
            nc.sync.dma_start(out_stats.ap()[:, 0:1], negm[:])
            nc.sync.dma_start(out_stats.ap()[:, 1:2], s_t[:])

    nc.compile()
    return nc


def _get_program(blocks, stage="full"):
    key = (blocks, stage)
    if key not in _PROG_CACHE:
        _PROG_CACHE[key] = _build_program(blocks, stage)
    return _PROG_CACHE[key]


def _prep(inputs):
    """Host-side sharding/sorting/packing. Returns (blocks, in_maps)."""
    x = np.asarray(inputs["paths_nodes"], dtype=np.float32)
    lengths = np.asarray(inputs["lengths"], dtype=np.int32)
    pW1 = np.asarray(inputs["pW1"], dtype=np.float32)
    pb1 = np.asarray(inputs["pb1"], dtype=np.float32)
    pw2 = np.asarray(inputs["pw2"], dtype=np.float32)
    aW1 = np.asarray(inputs["aW1"], dtype=np.float32)
    ab1 = np.asarray(inputs["ab1"], dtype=np.float32)
    aw2 = np.asarray(inputs["aw2"], dtype=np.float32)
    # pb2 / ab2 shift their softmax logits uniformly -> no effect on output.

    bf = ml_dtypes.bfloat16
    len_sh = lengths.reshape(NCORES, PS)
    orders = np.argsort(-len_sh, axis=1, kind="stable")        # [NC, PS] desc
    sorted_len = np.take_along_axis(len_sh, orders, axis=1)
    len_max = sorted_len.max(axis=0)                           # [PS]
    blocks = _make_blocks(len_max)

    x_sh = x.reshape(NCORES, PS, LMAX, D)
    w1_np = np.ascontiguousarray(pW1.reshape(KC, 128, H)).astype(np.float32)
    w2_np = np.ascontiguousarray(pw2.reshape(HC, 128).T).astype(np.float32)
    b1_np = np.ascontiguousarray(pb1.reshape(HC, 128).T).astype(np.float32)
    aw1_np = np.ascontiguousarray(aW1.reshape(KC, 128, H)).astype(np.float32)
    ab1_np = np.ascontiguousarray(ab1.reshape(HC, 128).T).astype(np.float32)
    aw2_np = np.ascontiguousarray(aw2.reshape(HC, 128).T).astype(np.float32)
    one1 = np.ones((1, 1), dtype=bf)

    ar = np.arange(LMAX)
    in_maps = []
    for c in range(NCORES):
        xc = x_sh[c][orders[c]]                       # [PS, LMAX, D] sorted
        lc = sorted_len[c]                            # [PS]
        xr_parts = []
        mk_parts = []
        p = 0
        for (bp, cap) in blocks:
            xblk = xc[p : p + bp, :cap, :]            # [bp, cap, D]
            xb_t = (
                xblk.reshape(bp, cap, KC, 128)
                .transpose(2, 3, 0, 1)
                .reshape(KC, 128, bp * cap)
            )
            xr_parts.append(xb_t.astype(bf).ravel())
            lb = lc[p : p + bp]
            mk = np.where(ar[None, :cap] < lb[:, None], 0.0, MASK_NEG)
            mk_parts.append(mk.astype(bf).ravel())
            p += bp
        in_maps.append(
            {
                "xb": np.concatenate(xr_parts),
                "msk": np.concatenate(mk_parts),
                "w1": w1_np,
                "w2": w2_np,
                "b1": b1_np,
                "aw1": aw1_np,
                "ab1": ab1_np,
                "aw2": aw2_np,
                "one1_bf": one1,
            }
        )
    return blocks, in_maps


def kernel(**inputs):
    global LAST_RESULT
    blocks, in_maps = _prep(inputs)
    nc = _get_program(blocks)

    res = bass_utils.run_bass_kernel_spmd(
        nc, in_maps, core_ids=list(range(NCORES)), **_TRACE_KW
    )
    LAST_RESULT = res

    parts = np.stack([r["out_part"] for r in res.results])    # [8, 128, KC]
    stats = np.stack([r["out_stats"] for r in res.results])   # [8, 1, 2]
    m = -stats[:, 0, 0]
    s = stats[:, 0, 1]
    mg = m.max()
    sc = np.exp(m - mg)
    total = float((sc * s).sum())
    vec = (parts * sc[:, None, None]).sum(axis=0)             # [128, KC]
    user = np.ascontiguousarray(vec.T).reshape(D) / total
    return user.astype(np.float32)
